# revision 38
# baseline (speedup 1.0000x reference)
"""Trainium2 Bass kernel for nn_DynamicHybridModulation.

Sharding: data-parallel over batch (B=8 -> 8 cores, one batch each).  The
only cross-core communication is a 6-float AllReduce for the global
BatchNorm statistics of the bias branch.

Math bookkeeping: the kernel computes S'' = 16*scores_ref via ternary
q'' = -spike(q_lin) and k'' = -(spike(k_lin) + k_lin) (the two minus signs
cancel in the product; the negated forms need one fewer DVE op).  The /16
is folded into the t-threshold (>=16), the exp scale (1/16) and the sw gate
(sw16 = 16*K_BIAS*sw).  Q/K/V biases ride on an augmented contraction row
(row 768 of the padded weights = bias, ones row in hs^T).

Stats trick: Sign(S''-16) summed over an axis gives  #above - #below, and
count_above = (sum + 512)/2.  That affine fixup is linear, so it is folded
into the conv1 weights (C/1024) plus a per-channel constant
c0 = 0.5*sum_h C[r,h] applied as an ACT bias when copying conv1's output
out of PSUM.  One Scalar-engine Sign pass per score tile therefore yields
both the row stats (accum_out) and the tile fed to the column-stat
ones-matmul, keeping the Vector engine free for the phase-C gate ops.

Scores are never stored: phase C recomputes them on the PE (cheaper than a
PSUM->SBUF cast on the DVE, and it keeps the tensor engine warm).  The
attention mask is applied only in the phase-C recompute (reference
semantics: the t stats use pre-mask scores); for the all-zeros mask of this
problem the mask matmul is skipped entirely.
"""

import numpy as np

try:
    import concourse  # noqa: F401
except ImportError:  # pragma: no cover
    import sys

    for p in ("/opt/trn_rl_repo", "/root/.axon_site/_ro/trn_rl_repo"):
        sys.path.insert(0, p)

import concourse.bass as bass  # noqa: E402,F401
import concourse.tile as tile  # noqa: E402
from concourse import bacc, mybir  # noqa: E402
from concourse.bass_utils import run_bass_kernel_spmd  # noqa: E402

F32 = mybir.dt.float32
F32R = mybir.dt.float32r
BF16 = mybir.dt.bfloat16
F16 = mybir.dt.float16
ALU = mybir.AluOpType
ACTF = mybir.ActivationFunctionType

B, S, DM, H, D, R = 8, 512, 768, 12, 64, 3
NT = DM // 128  # 6 dout tiles
KT = S // 128  # 4 s tiles
NI = NT + 1  # 7 contraction tiles (6 x 128 + bias row block)
N_TOT = float(B * 2 * S)

_CACHE = {}


def _round_fp32r(x):
    """Round fp32 to the 11-explicit-mantissa-bit grid the PE uses for
    float32r operands (calibrated against hardware)."""
    u = np.ascontiguousarray(x, np.float32).view(np.uint32).copy()
    u = (u + np.uint32(0x800)) & np.uint32(0xFFFFF000)
    return u.view(np.float32)


def _build(mask_nonzero, bias_nonzero):
    nc = bacc.Bacc("TRN2", target_bir_lowering=False, debug=False, num_devices=8)

    hsT_d = nc.dram_tensor("hsT", [128, NI, S], F32R, kind="ExternalInput").ap()
    wq_d = nc.dram_tensor("wq", [NT, 128, NI, 128], F32R, kind="ExternalInput").ap()
    wk_d = nc.dram_tensor("wk", [NT, 128, NI, 128], F32R, kind="ExternalInput").ap()
    wv_d = nc.dram_tensor("wv", [128, NI, DM], F32R, kind="ExternalInput").ap()
    mask_d = (
        nc.dram_tensor("mask_rows", [1, S], F32R, kind="ExternalInput").ap()
        if mask_nonzero
        else None
    )
    conv1T_d = nc.dram_tensor("conv1T", [H, R], F32R, kind="ExternalInput").ap()
    c0_d = nc.dram_tensor("c0", [R, 1], F32, kind="ExternalInput").ap()
    convhT_d = nc.dram_tensor("convhT", [R, H], F32R, kind="ExternalInput").ap()
    convwT_d = nc.dram_tensor("convwT", [R, H], F32R, kind="ExternalInput").ap()
    gamma_d = nc.dram_tensor("gamma", [R, 1], F32, kind="ExternalInput").ap()
    beta_d = nc.dram_tensor("beta", [R, 1], F32, kind="ExternalInput").ap()
    outT_d = nc.dram_tensor("outT", [DM, S], F32, kind="ExternalOutput").ap()
    ar_in_d = nc.dram_tensor("ar_bounce", [R, 2], F32).ap()
    ar_out_d = nc.dram_tensor("ar_shared", [R, 2], F32, addr_space="Shared").ap()

    ni = NI if bias_nonzero else NT  # skip the bias row pass when all-zero
    with tile.TileContext(nc) as tc:
        with (
            tc.tile_pool(name="const", bufs=1) as cpool,
            tc.tile_pool(name="wstream", bufs=3) as wpool,
            tc.tile_pool(name="big", bufs=1) as bigpool,
            tc.tile_pool(name="ctxs", bufs=12) as ctxpool,
            tc.tile_pool(name="wk3", bufs=4) as wk3pool,
            tc.tile_pool(name="wk2", bufs=2) as wk2pool,
            tc.tile_pool(name="sgn", bufs=8) as sgnpool,
            tc.tile_pool(name="shbp", bufs=4) as shbpool,
            tc.tile_pool(name="prep", bufs=4) as prepool,
            tc.tile_pool(name="ebuf", bufs=4) as epool,
            tc.tile_pool(name="ps", bufs=5, space="PSUM") as pspool,
            tc.tile_pool(name="ps2", bufs=3, space="PSUM") as ps2pool,
        ):
            # ---- resident loads (partition-major, contiguous per line) ----
            hsT_t = cpool.tile([128, NI, S], F32R)
            for i in range(NI):
                nc.sync.dma_start(hsT_t[:, i, :], hsT_d[:, i, :])
            wv_t = cpool.tile([128, NI, DM], F32R)
            if mask_nonzero:
                mask_t = cpool.tile([1, S], F32R)
                nc.sync.dma_start(mask_t[:], mask_d[:])
            conv1T_t = cpool.tile([H, R], F32R)
            nc.sync.dma_start(conv1T_t[:], conv1T_d[:])
            c0_t = cpool.tile([R, 1], F32)
            nc.sync.dma_start(c0_t[:], c0_d[:])
            convhT_t = cpool.tile([R, H], F32R)
            nc.sync.dma_start(convhT_t[:], convhT_d[:])
            convwT_t = cpool.tile([R, H], F32R)
            nc.sync.dma_start(convwT_t[:], convwT_d[:])
            gamma_t = cpool.tile([R, 1], F32)
            nc.sync.dma_start(gamma_t[:], gamma_d[:])
            beta_t = cpool.tile([R, 1], F32)
            nc.sync.dma_start(beta_t[:], beta_d[:])

            ones_f16 = cpool.tile([128, 1], F16)
            nc.gpsimd.memset(ones_f16[:], 1.0)
            neg16 = cpool.tile([128, 1], F32)
            nc.gpsimd.memset(neg16[:], -16.0)
            if mask_nonzero:
                ones_q = cpool.tile([1, S], F32)
                nc.gpsimd.memset(ones_q[:], 1.0)
                ones_q_r = cpool.tile([1, S], F32R)
                nc.scalar.copy(ones_q_r[:], ones_q[:])

            # ---- persistent intermediates ----
            qT_t = bigpool.tile([128, NT, S], F32R)  # -spike(q_lin)^T
            kT_t = bigpool.tile([128, NT, S], F32R)  # -(spike+lin)(k_lin)^T
            v_t = bigpool.tile([128, KT, H * 65], BF16)  # v with ones cols
            cat_t = bigpool.tile([H, 2 * S], F32R)  # sign-sums [xh | xw]
            xw_cols = bigpool.tile([128, H * KT], F32R)  # sign accum slots
            sw16_cols = bigpool.tile([128, KT, H], F32)  # 16*sigmoid(convw)
            sh_t = bigpool.tile([H, S], F16)
            NB = 3  # reciprocal batches
            HB = H // NB
            den_cols = bigpool.tile([128, H * KT], F16)  # denom, partition-major
            rec_cols = bigpool.tile([128, H * KT], F16)

            for st in range(KT):
                nc.gpsimd.memset(
                    v_t[:, st, :].rearrange("p (h c) -> p h c", c=65)[:, :, 64:65], 1.0
                )

            # =========== PHASE A: projections ===========
            for j in range(NT):
                for proj, w_d, dst in (("q", wq_d, qT_t), ("k", wk_d, kT_t)):
                    w_t = wpool.tile([128, NI, 128], F32R, tag="wblk")
                    for c in range(4):
                        nc.sync.dma_start(
                            w_t[c * 32 : (c + 1) * 32, :, :],
                            w_d[j][c * 32 : (c + 1) * 32],
                        )
                    pa = pspool.tile([128, S], F32, tag="ps")
                    for i in range(ni):
                        nc.tensor.matmul(
                            pa[:],
                            w_t[:, i, :],
                            hsT_t[:, i, :],
                            start=(i == 0),
                            stop=(i == ni - 1),
                        )
                    if proj == "q":
                        # q''n = (x<=-1) - (x>=1) = -spike(x)
                        t1 = wk3pool.tile([128, S], F32, tag="qk_tmp")
                        nc.vector.tensor_scalar(t1[:], pa[:], 1.0, None, ALU.is_ge)
                        nc.vector.scalar_tensor_tensor(
                            dst[:, j, :], pa[:], -1.0, t1[:], ALU.is_le, ALU.subtract
                        )
                    else:
                        # k''n = (x<=-1) - (x>=1) - x = -(spike(x) + x)
                        t1 = wk3pool.tile([128, S], F32, tag="qk_tmp")
                        nc.vector.tensor_scalar(
                            t1[:], pa[:], 1.0, -1.0, ALU.is_ge, ALU.mult
                        )
                        t2 = wk3pool.tile([128, S], F32, tag="qk_tmp2")
                        nc.vector.scalar_tensor_tensor(
                            t2[:], pa[:], -1.0, t1[:], ALU.is_le, ALU.add
                        )
                        nc.vector.tensor_tensor(
                            dst[:, j, :], t2[:], pa[:], ALU.subtract
                        )

            # =========== PHASE B: scores + sign stats ===========
            with nc.allow_low_precision(reason="sign sums are small integers"):
                for h in range(H):
                    jh, p0 = divmod(h * D, 128)
                    sgs = []
                    for kt in range(KT):
                        ps = pspool.tile([128, S], F32, tag="ps")
                        nc.tensor.matmul(
                            ps[:],
                            kT_t[p0 : p0 + D, jh, kt * 128 : (kt + 1) * 128],
                            qT_t[p0 : p0 + D, jh, :],
                            start=True,
                            stop=True,
                        )
                        sg = sgnpool.tile([128, S], F16, tag="sgn")
                        if h < H // 2:
                            nc.scalar.activation(
                                sg[:],
                                ps[:],
                                ACTF.Sign,
                                bias=neg16[:],
                                accum_out=xw_cols[:, h * KT + kt : h * KT + kt + 1],
                            )
                        else:
                            nc.vector.tensor_scalar(
                                sg[:],
                                ps[:],
                                16.0,
                                None,
                                ALU.is_ge,
                                ALU.add,
                                accum_out=xw_cols[:, h * KT + kt : h * KT + kt + 1],
                            )
                        sgs.append(sg)
                    # sum the four stat tiles on the DVE (idle in this phase)
                    # so the column-stat ones-matmul runs once per head
                    sa = wk3pool.tile([128, S], F16, tag="sgsuma")
                    nc.vector.tensor_tensor(sa[:], sgs[0][:], sgs[1][:], ALU.add)
                    sb = wk3pool.tile([128, S], F16, tag="sgsumb")
                    nc.vector.tensor_tensor(sb[:], sgs[2][:], sgs[3][:], ALU.add)
                    sc = wk3pool.tile([128, S], F16, tag="sgsumc")
                    nc.vector.tensor_tensor(sc[:], sa[:], sb[:], ALU.add)
                    pxh = ps2pool.tile([1, S], F32, tag="pacc")
                    nc.tensor.matmul(
                        pxh[:], ones_f16[:], sc[:], start=True, stop=True
                    )
                    xh_row = wk2pool.tile([1, S], F32R, tag="xhrow")
                    nc.scalar.copy(xh_row[:], pxh[:])
                    nc.sync.dma_start(cat_t[h : h + 1, 0:S], xh_row[:])
                    for kt in range(KT):
                        nc.sync.dma_start(
                            cat_t[h : h + 1, S + kt * 128 : S + (kt + 1) * 128],
                            xw_cols[:, h * KT + kt : h * KT + kt + 1],
                        )

            # =========== MID: conv1 -> BN(allreduce) -> gates ===========
            pyh = pspool.tile([R, S], F32, tag="ps")
            pyw = pspool.tile([R, S], F32, tag="ps")
            nc.tensor.matmul(pyh[:], conv1T_t[:], cat_t[:, 0:S], start=True, stop=True)
            nc.tensor.matmul(pyw[:], conv1T_t[:], cat_t[:, S:], start=True, stop=True)
            y_t = bigpool.tile([R, 2 * S], F32)
            # + c0: folds the (sign_sum + 512)/2 affine fixup of both stats
            nc.scalar.activation(y_t[:, :S], pyh[:], ACTF.Identity, bias=c0_t[:])
            nc.scalar.activation(y_t[:, S:], pyw[:], ACTF.Identity, bias=c0_t[:])

            stats_t = bigpool.tile([R, 2], F32)
            nc.vector.tensor_reduce(
                stats_t[:, 0:1], y_t[:], mybir.AxisListType.X, ALU.add
            )
            yn_t = bigpool.tile([R, 2 * S], F32)
            nc.vector.tensor_tensor(yn_t[:], y_t[:], y_t[:], ALU.mult)
            nc.vector.tensor_reduce(
                stats_t[:, 1:2], yn_t[:], mybir.AxisListType.X, ALU.add
            )
            nc.sync.dma_start(ar_in_d[:], stats_t[:])
            nc.gpsimd.collective_compute(
                "AllReduce",
                ALU.add,
                replica_groups=[list(range(8))],
                ins=[ar_in_d[:]],
                outs=[ar_out_d[:]],
            )
            for c in range(4):
                nc.sync.dma_start(
                    wv_t[c * 32 : (c + 1) * 32, :, :], wv_d[c * 32 : (c + 1) * 32]
                )
            for st in range(KT):
                for dh in range(2):
                    pv = pspool.tile([128, S], F32, tag="ps")
                    for i in range(ni):
                        nc.tensor.matmul(
                            pv[:, :384],
                            hsT_t[:, i, st * 128 : (st + 1) * 128],
                            wv_t[:, i, dh * 384 : (dh + 1) * 384],
                            start=(i == 0),
                            stop=(i == ni - 1),
                        )
                    dst = v_t[:, st, dh * 6 * 65 : (dh + 1) * 6 * 65].rearrange(
                        "p (h c) -> p h c", c=65
                    )[:, :, 0:64]
                    nc.scalar.copy(
                        dst, pv[:, :384].rearrange("p (h c) -> p h c", c=64)
                    )

            gstats_t = bigpool.tile([R, 2], F32)
            nc.sync.dma_start(gstats_t[:], ar_out_d[:])

            mom_t = bigpool.tile([R, 2], F32)
            nc.vector.tensor_scalar(
                mom_t[:], gstats_t[:], 1.0 / N_TOT, None, ALU.mult
            )
            mu_t = mom_t[:, 0:1]
            ex2_t = mom_t[:, 1:2]
            nvar_t = bigpool.tile([R, 1], F32)
            nc.vector.scalar_tensor_tensor(
                nvar_t[:], mu_t[:], mu_t[:], ex2_t[:], ALU.mult, ALU.subtract
            )
            vpe_t = bigpool.tile([R, 1], F32)
            nc.vector.tensor_scalar(vpe_t[:], nvar_t[:], -1.0, 1e-5, ALU.mult, ALU.add)
            sd_t = bigpool.tile([R, 1], F32)
            nc.scalar.sqrt(sd_t[:], vpe_t[:])
            inv_t = bigpool.tile([R, 1], F32)
            nc.vector.reciprocal(inv_t[:], sd_t[:])
            gp_t = bigpool.tile([R, 1], F32)
            nc.vector.tensor_tensor(gp_t[:], gamma_t[:], inv_t[:], ALU.mult)
            mg_t = bigpool.tile([R, 1], F32)
            nc.vector.tensor_tensor(mg_t[:], mu_t[:], gp_t[:], ALU.mult)
            bp_t = bigpool.tile([R, 1], F32)
            nc.vector.tensor_tensor(bp_t[:], beta_t[:], mg_t[:], ALU.subtract)
            nc.vector.tensor_scalar(
                yn_t[:], y_t[:], gp_t[:], bp_t[:], ALU.mult, ALU.add
            )
            yr_t = bigpool.tile([R, 2 * S], F32R)
            nc.scalar.activation(yr_t[:], yn_t[:], ACTF.Relu)

            psh = pspool.tile([H, S], F32, tag="ps")
            nc.tensor.matmul(psh[:], convhT_t[:], yr_t[:, :S], start=True, stop=True)
            sh_sig = bigpool.tile([H, S], F16)
            nc.scalar.activation(sh_sig[:], psh[:], ACTF.Sigmoid)
            nc.vector.tensor_scalar(sh_t[:], sh_sig[:], 16.0, None, ALU.mult)
            for st in range(KT):
                psw = pspool.tile([128, H], F32, tag="ps")
                nc.tensor.matmul(
                    psw[:],
                    yr_t[:, S + st * 128 : S + (st + 1) * 128],
                    convwT_t[:],
                    start=True,
                    stop=True,
                )
                nc.scalar.activation(sw16_cols[:, st, :], psw[:], ACTF.Sigmoid)

            # =========== PHASE C: bias + softmax + context ===========
            ctx_stages = []
            for h in range(H):
                jh, p0 = divmod(h * D, 128)
                sh_stage = wk2pool.tile([1, S], F16, tag="shstage")
                nc.gpsimd.dma_start(sh_stage[:], sh_t[h : h + 1, :])
                shb = shbpool.tile([128, S], F16, tag="shb")
                nc.gpsimd.partition_broadcast(shb[:], sh_stage[:])
                pre_t = prepool.tile([128, KT, S], F16, tag="pre")
                pctx = ps2pool.tile([65, S], F32, tag="pacc")
                for kt in range(KT):
                    ps = pspool.tile([128, S], F32, tag="ps")
                    nc.tensor.matmul(
                        ps[:],
                        kT_t[p0 : p0 + D, jh, kt * 128 : (kt + 1) * 128],
                        qT_t[p0 : p0 + D, jh, :],
                        start=True,
                        stop=not mask_nonzero,
                    )
                    if mask_nonzero:
                        nc.tensor.matmul(
                            ps[:],
                            mask_t[:, kt * 128 : (kt + 1) * 128],
                            ones_q_r[:],
                            start=False,
                            stop=True,
                        )
                    if h % 3 == 1:
                        # ACT path: t*sw via Sign then scaled Relu, then the
                        # sh16 product and the psum add on the DVE
                        tsg = wk3pool.tile([128, S], F16, tag="ctsg")
                        nc.scalar.activation(tsg[:], ps[:], ACTF.Sign, bias=neg16[:])
                        tsw = wk3pool.tile([128, S], F16, tag="ctsw")
                        nc.scalar.activation(
                            tsw[:],
                            tsg[:],
                            ACTF.Relu,
                            scale=sw16_cols[:, kt, h : h + 1],
                        )
                        tmp = wk3pool.tile([128, S], F16, tag="ctmp")
                        nc.vector.tensor_tensor(tmp[:], tsw[:], shb[:], ALU.mult)
                        nc.vector.tensor_tensor(
                            pre_t[:, kt, :], tmp[:], ps[:], ALU.add
                        )
                    else:
                        tmp = wk3pool.tile([128, S], F16, tag="ctmp")
                        nc.vector.scalar_tensor_tensor(
                            tmp[:], ps[:], 16.0, shb[:], ALU.is_ge, ALU.mult
                        )
                        nc.vector.scalar_tensor_tensor(
                            pre_t[:, kt, :],
                            tmp[:],
                            sw16_cols[:, kt, h : h + 1],
                            ps[:],
                            ALU.mult,
                            ALU.add,
                        )
                e_t = epool.tile([128, KT, S], BF16, tag="ebuf")
                for kt in range(KT):
                    nc.scalar.activation(
                        e_t[:, kt, :],
                        pre_t[:, kt, :],
                        ACTF.Exp,
                        scale=1.0 / 16.0,
                    )
                    nc.tensor.matmul(
                        pctx[:],
                        v_t[:, kt, h * 65 : (h + 1) * 65],
                        e_t[:, kt, :],
                        start=(kt == 0),
                        stop=(kt == KT - 1),
                    )
                ctx_stage = ctxpool.tile([D + 1, S], F16, tag="ctxs")
                nc.scalar.copy(ctx_stage[:], pctx[:])
                half, hh = divmod(h, HB)
                # scatter the denom row into partition-major columns so the
                # reciprocal runs wide (128 lanes x 16 deep, not 1 x 512)
                nc.gpsimd.dma_start(
                    den_cols[:, h * KT : (h + 1) * KT],
                    ctx_stage[D : D + 1, :].rearrange("o (kt p) -> o kt p", p=128),
                )
                ctx_stages.append(ctx_stage)
                if hh == HB - 1:
                    cs = slice(half * HB * KT, (half + 1) * HB * KT)
                    with nc.allow_low_precision(
                        reason="softmax denom reciprocal at fp16"
                    ):
                        nc.vector.reciprocal(rec_cols[:, cs], den_cols[:, cs])
                    for h2 in range(half * HB, (half + 1) * HB):
                        r_stage = wk2pool.tile([1, S], F16, tag="rstage")
                        nc.gpsimd.dma_start(
                            r_stage[:].rearrange("o (kt p) -> o kt p", p=128),
                            rec_cols[:, h2 * KT : (h2 + 1) * KT],
                        )
                        r_b = wk2pool.tile([D, S], F16, tag="rb")
                        nc.gpsimd.partition_broadcast(r_b[:], r_stage[:])
                        outp = wk2pool.tile([D, S], F32, tag="outp")
                        nc.vector.tensor_tensor(
                            outp[:], ctx_stages[h2][0:D, :], r_b[:], ALU.mult
                        )
                        nc.scalar.dma_start(
                            outT_d[h2 * D : (h2 + 1) * D, :], outp[:]
                        )

    nc.compile()
    return nc


def _prep_inputs(
    hidden_states,
    attention_mask,
    Wq,
    bq,
    Wk,
    bk,
    Wv,
    bv,
    conv1_w,
    bn_gamma,
    bn_beta,
    convh_w,
    convw_w,
):
    f32 = np.float32

    def pad_w(W, b):
        Wp = np.zeros((NI * 128, DM), f32)
        Wp[:DM] = _round_fp32r(np.asarray(W, f32))
        Wp[DM] = _round_fp32r(np.asarray(b, f32))
        return Wp

    def col_blocks(Wp):
        # [6(j), 128(p), 7(i), 128(c)]: per-partition contiguous DMA lines
        return np.ascontiguousarray(
            Wp.reshape(NI, 128, NT, 128).transpose(2, 1, 0, 3)
        )

    wq_p = col_blocks(pad_w(Wq, bq))
    wk_p = col_blocks(pad_w(Wk, bk))
    wv_p = np.ascontiguousarray(
        pad_w(Wv, bv).reshape(NI, 128, DM).transpose(1, 0, 2)
    )
    conv1 = np.asarray(conv1_w, f32)
    # heads 0..5 produce sign-sums (affine fixup), heads 6..11 raw counts
    scale_h = np.where(np.arange(H) < H // 2, 1.0 / (2.0 * S), 1.0 / S)
    conv1T = np.ascontiguousarray(_round_fp32r(conv1.T * scale_h[:, None]))
    c0 = np.ascontiguousarray(
        (0.5 * conv1[:, : H // 2].sum(axis=1)).reshape(R, 1).astype(f32)
    )
    convhT = np.ascontiguousarray(_round_fp32r(np.asarray(convh_w, f32).T))
    convwT = np.ascontiguousarray(_round_fp32r(np.asarray(convw_w, f32).T))
    gamma = np.asarray(bn_gamma, f32).reshape(R, 1)
    beta = np.asarray(bn_beta, f32).reshape(R, 1)

    hs = np.asarray(hidden_states, f32)
    am = np.asarray(attention_mask, f32)
    in_maps = []
    for b in range(B):
        hsT = np.zeros((NI * 128, S), f32)
        hsT[:DM] = _round_fp32r(hs[b].T)
        hsT[DM] = 1.0
        hsT_p = np.ascontiguousarray(hsT.reshape(NI, 128, S).transpose(1, 0, 2))
        mask_rows = np.ascontiguousarray(_round_fp32r(am[b, 0, 0]).reshape(1, S))
        extra = {"mask_rows": mask_rows} if np.any(am) else {}
        in_maps.append(
            dict(
                hsT=hsT_p,
                wq=wq_p,
                wk=wk_p,
                wv=wv_p,
                **extra,
                conv1T=conv1T,
                c0=c0,
                convhT=convhT,
                convwT=convwT,
                gamma=gamma,
                beta=beta,
            )
        )
    return in_maps


def _run(inputs, trace=False, trace_kwargs=None):
    mask_nonzero = bool(np.any(np.asarray(inputs["attention_mask"])))
    bias_nonzero = any(
        bool(np.any(np.asarray(inputs[k]))) for k in ("bq", "bk", "bv")
    )
    key = ("nc", mask_nonzero, bias_nonzero)
    if key not in _CACHE:
        _CACHE[key] = _build(mask_nonzero, bias_nonzero)
    nc = _CACHE[key]
    in_maps = _prep_inputs(**inputs)
    res = run_bass_kernel_spmd(
        nc, in_maps, list(range(8)), trace=trace, **(trace_kwargs or {})
    )
    out = np.stack([np.ascontiguousarray(r["outT"].T) for r in res.results])
    return out, res


def kernel(**inputs):
    out, _ = _run(inputs, trace=False)
    return out
